# revision 1
# baseline (speedup 1.0000x reference)
"""Trainium2 Bass kernel for the autoregressive GRU decoder.

Reference computation (eval-mode Decoder):
  x0 = x[:, 30, :]                # only element of x ever used
  h0 = h[0]
  for t in 0..29:
      h = GRUCell(x_t, h)         # PyTorch gate layout [r, z, n]
      y_t = h @ W_out.T + b_out
      x_{t+1} = y_t               # linear feedback -> fold into weights
  out = stack(y_t)                # [B, 30, 32]

Because the feedback x_{t+1} = W_out @ h_t + b_out is linear, for t >= 1:
  gi_t = W_ih @ x_t + b_ih = (W_ih @ W_out) @ h_{t-1} + (W_ih @ b_out + b_ih)
so every step t >= 1 is a pure H->H recurrence; weights are folded on the
host and the r/z gates use a single combined matrix (W_hh + W_ih_eff).

Kernel shape notes:
  - state h^T transposed [H=128 partitions, Bc=2048 free], updated in
    place (the h' write happens after the last h read of the step), so
    the whole recurrence runs as ONE hardware For_i loop over steps with
    step 0 peeled (it consumes x0 with the unfolded weights).
  - one PSUM tile [128, 4096] (all 8 banks); gate matmuls grouped by
    gate so each consumer (sigmoid / scalar_tensor_tensor) covers the
    full batch in one dense op.  N=512 per matmul (PSUM-bank ISA limit).
  - y_t computed transposed ([32, batch]) with W_out^T stationary; bias
    folds into the PSUM->SBUF copy; per-step DMA with a loop-register
    DRAM offset into [STEPS, I*Bc]; host transposes once at the end.
  - f32r end-to-end for all matmul operands (full-rate PE, ~1e-4 err).

Sharding: pure data parallel over batch, 8 cores x 2048, no collectives.
"""

import os

import numpy as np

B, T, I, H, SEQLEN = 16384, 60, 32, 128, 30
STEPS = T - SEQLEN  # 30
NCORES = 8
BC = B // NCORES  # 2048 batch rows per core
MMN = 512  # matmul moving-operand free-dim limit (one PSUM bank of fp32)

LOOP = os.environ.get("K_LOOP", "1") == "1"

LAST_RESULT = None  # BassKernelResults of the most recent run (for test.py)

_CACHE = {}


def _build(repeats=1, loop=LOOP):
    from contextlib import ExitStack

    import concourse.bacc as bacc
    import concourse.bass as bass
    import concourse.mybir as mybir
    import concourse.tile as tile

    f32 = mybir.dt.float32
    f32r = mybir.dt.float32r
    Alu = mybir.AluOpType
    Act = mybir.ActivationFunctionType

    nc = bacc.Bacc()

    # packed inputs (fewer DMAs -> fewer instructions and wait sources):
    # cst  [H, 4H | 2H | I | 8]  = WA | WA0 | WoutT | BIAS(bitcast f32)
    # xw   [I, BC | 3H]          = x0^T | W0
    # h0t  [H, BC]
    CW = 4 * H + 2 * H + I + 8
    dcst = nc.dram_tensor("cst", [H, CW], f32r, kind="ExternalInput")
    dxw = nc.dram_tensor("xw", [I, BC + 3 * H], f32r, kind="ExternalInput")
    dh = nc.dram_tensor("h0t", [H, BC], f32r, kind="ExternalInput")
    dout = nc.dram_tensor("out", [STEPS, I * BC], f32, kind="ExternalOutput")

    with ExitStack() as ctx:
        tc = ctx.enter_context(tile.TileContext(nc))
        const = ctx.enter_context(tc.tile_pool(name="const", bufs=1))
        work = ctx.enter_context(tc.tile_pool(name="work", bufs=1))
        psum = ctx.enter_context(tc.tile_pool(name="psum", bufs=1, space="PSUM"))

        def load_const(dram, shape, name):
            t = const.tile(shape, dram.dtype, tag=name)
            nc.sync.dma_start(out=t[:], in_=dram[:, :])
            return t

        scst = load_const(dcst, [H, CW], "cst")
        sxw = load_const(dxw, [I, BC + 3 * H], "xw")
        hT = load_const(dh, [H, BC], "h")  # the state, updated in place

        sbias = scst[:, 6 * H + I : 6 * H + I + 8].bitcast(f32)
        b_r = sbias[:, 0:1]
        b_z = sbias[:, 1:2]
        b_hn = sbias[:, 2:3]
        b_in = sbias[:, 3:4]
        b0_r = sbias[:, 4:5]
        b0_z = sbias[:, 5:6]
        b0_in = sbias[:, 6:7]
        b_y = sbias[0:I, 7:8]  # b_out padded into rows 0..31

        A_r = scst[:, 0 * H : 1 * H]
        A_z = scst[:, 1 * H : 2 * H]
        A_hn = scst[:, 2 * H : 3 * H]
        A_in = scst[:, 3 * H : 4 * H]
        A0_r = scst[:, 4 * H : 5 * H]
        A0_z = scst[:, 5 * H : 6 * H]
        WoutT = scst[:, 6 * H : 6 * H + I]
        sx0 = sxw[:, 0:BC]
        W0_r = sxw[:, BC + 0 * H : BC + 1 * H]
        W0_z = sxw[:, BC + 1 * H : BC + 2 * H]
        W0_n = sxw[:, BC + 2 * H : BC + 3 * H]

        def quad_mm(P, col, A, rhs_full, extra=None):
            """Fill P[:, col*2048 : +2048] with A.T @ rhs_full (N=512 x4).

            extra = (W0_g, x0) accumulates the step-0 input term."""
            base = col * BC
            for q in range(4):
                sl = slice(q * MMN, (q + 1) * MMN)
                dst = P[:, base + q * MMN : base + (q + 1) * MMN]
                nc.tensor.matmul(dst, A, rhs_full[:, sl],
                                 start=True, stop=extra is None)
                if extra is not None:
                    nc.tensor.matmul(dst, extra[0], extra[1][:, sl],
                                     start=False, stop=True)

        def gru_step(first, y_dst):
            """One GRU step, state updated in place; y DMA'd to y_dst AP."""
            P = psum.tile([128, 2 * BC], f32, tag="P", bufs=1)

            if first:
                xr, xz = (W0_r, sx0), (W0_z, sx0)
                cb_r, cb_z, cb_in = b0_r, b0_z, b0_in
            else:
                xr = xz = None
                cb_r, cb_z, cb_in = b_r, b_z, b_in

            # phase A: r/z gates, full batch, grouped by gate
            quad_mm(P, 0, A0_r if first else A_r, hT, xr)
            quad_mm(P, 1, A0_z if first else A_z, hT, xz)
            r_sb = work.tile([128, BC], f32, tag="r")
            z_sb = work.tile([128, BC], f32, tag="z")
            nc.scalar.activation(r_sb[:], P[:, 0:BC], Act.Sigmoid, bias=cb_r)
            nc.scalar.activation(z_sb[:], P[:, BC:], Act.Sigmoid, bias=cb_z)

            # phase B: hn/in gates reuse the same PSUM banks.
            # at t=0 the input-gate term is W_ih_n @ x0 only (the folded
            # A_in matrix encodes the y->x feedback, which starts at t=1)
            quad_mm(P, 0, A_hn, hT)
            if first:
                quad_mm(P, 1, W0_n, sx0)
            else:
                quad_mm(P, 1, A_in, hT)
            # u = (g_hn + b_hn) * r ; v = (g_in + b_in) + u  (in place)
            u_sb = work.tile([128, BC], f32, tag="u")
            nc.vector.scalar_tensor_tensor(
                u_sb[:], P[:, 0:BC], b_hn, r_sb[:], Alu.add, Alu.mult
            )
            nc.vector.scalar_tensor_tensor(
                u_sb[:], P[:, BC:], cb_in, u_sb[:], Alu.add, Alu.add
            )
            n_sb = work.tile([128, BC], f32, tag="n")
            nc.scalar.activation(n_sb[:], u_sb[:], Act.Tanh)

            # phase C: h <- n + z * (h - n); the subtract is the last read
            # of the old state, so the final add may write h in place
            w_sb = work.tile([128, BC], f32, tag="w")
            nc.vector.tensor_tensor(w_sb[:], hT[:, :], n_sb[:], Alu.subtract)
            nc.vector.tensor_tensor(w_sb[:], z_sb[:], w_sb[:], Alu.mult)
            nc.vector.tensor_tensor(hT[:, :], n_sb[:], w_sb[:], Alu.add)

            # phase D: y^T = W_out @ h' into PSUM rows 0..31, bias+copy, DMA
            for q in range(4):
                sl = slice(q * MMN, (q + 1) * MMN)
                nc.tensor.matmul(P[0:I, q * MMN : (q + 1) * MMN],
                                 WoutT, hT[:, sl])
            y_sb = work.tile([I, BC], f32, tag="y")
            nc.vector.tensor_scalar_add(y_sb[:], P[0:I, 0:BC], b_y)
            nc.sync.dma_start(out=y_dst, in_=y_sb[:, None, :])

        for _rep in range(repeats):
            if _rep > 0:  # reload initial state for benchmarking repeats
                nc.sync.dma_start(out=hT[:], in_=dh[:, :])
            gru_step(True, dout[0:1, :].rearrange("o (p f) -> p o f", p=I))
            if loop:
                # staggered_reset: replaces the back-edge drain + two
                # all-engine barriers with overlapped per-stage semaphore
                # resets — measured 9.6 ms vs 24 ms per run, same result
                with tc.For_i(1, STEPS, 1, staggered_reset=True) as i:
                    gru_step(
                        False,
                        dout[bass.ds(i, 1), :].rearrange(
                            "o (p f) -> p o f", p=I
                        ),
                    )
            else:
                for t in range(1, STEPS):
                    gru_step(
                        False, dout[t : t + 1, :].rearrange("o (p f) -> p o f", p=I)
                    )

    return nc


def _host_prep(x, h, W_ih, W_hh, b_ih, b_hh, W_out, b_out):
    """Fold weights on the host (float64 for exactness), build per-core maps."""
    x = np.asarray(x, dtype=np.float32)
    h = np.asarray(h, dtype=np.float32)
    W_ih = np.asarray(W_ih, dtype=np.float64)
    W_hh = np.asarray(W_hh, dtype=np.float64)
    b_ih = np.asarray(b_ih, dtype=np.float64)
    b_hh = np.asarray(b_hh, dtype=np.float64)
    W_out = np.asarray(W_out, dtype=np.float64)
    b_out = np.asarray(b_out, dtype=np.float64)

    W_ih_eff = W_ih @ W_out  # [3H, H]
    b_ih_eff = W_ih @ b_out + b_ih  # [3H]

    def cvt(a):
        return np.ascontiguousarray(a, dtype=np.float32)

    WA = cvt(
        np.concatenate(
            [
                (W_hh[0:H] + W_ih_eff[0:H]).T,
                (W_hh[H : 2 * H] + W_ih_eff[H : 2 * H]).T,
                W_hh[2 * H : 3 * H].T,
                W_ih_eff[2 * H : 3 * H].T,
            ],
            axis=1,
        )
    )  # [H, 4H]
    WA0 = cvt(np.concatenate([W_hh[0:H].T, W_hh[H : 2 * H].T], axis=1))
    W0 = cvt(
        np.concatenate([W_ih[0:H].T, W_ih[H : 2 * H].T, W_ih[2 * H : 3 * H].T], axis=1)
    )  # [I, 3H]
    WoutT = cvt(W_out.T)  # [H, I]
    bx = np.zeros(H)
    bx[0:I] = b_out
    BIAS = cvt(
        np.stack(
            [
                b_hh[0:H] + b_ih_eff[0:H],
                b_hh[H : 2 * H] + b_ih_eff[H : 2 * H],
                b_hh[2 * H : 3 * H],
                b_ih_eff[2 * H : 3 * H],
                b_hh[0:H] + b_ih[0:H],
                b_hh[H : 2 * H] + b_ih[H : 2 * H],
                b_ih[2 * H : 3 * H],
                bx,
            ],
            axis=1,
        )
    )  # [H, 8]

    x0T = cvt(x[:, SEQLEN, :].T)  # [I, B]
    h0T = cvt(h[0].T)  # [H, B]

    CST = np.concatenate([WA, WA0, WoutT, BIAS], axis=1)  # [H, CW]
    in_maps = []
    for core in range(NCORES):
        cs = slice(core * BC, (core + 1) * BC)
        in_maps.append(
            {
                "cst": CST,
                "xw": np.concatenate(
                    [np.ascontiguousarray(x0T[:, cs]), W0], axis=1
                ),
                "h0t": np.ascontiguousarray(h0T[:, cs]),
            }
        )
    return in_maps


def _unshuffle(out_dev):
    """[STEPS, I*BC] device layout -> [BC, STEPS, I]."""
    x = out_dev.reshape(STEPS, I, BC)
    return np.ascontiguousarray(x.transpose(2, 0, 1))


def _get_nc(repeats=1):
    key = (repeats, LOOP)
    if key not in _CACHE:
        nc = _build(repeats)
        # Bacc needs explicit finalize (wait-splitting, reg alloc);
        # run_bass_via_pjrt serializes the module as-is.
        nc.finalize()
        _CACHE[key] = nc
    return _CACHE[key]


def run(in_maps, repeats=1):
    global LAST_RESULT
    from concourse.bass_utils import run_bass_kernel_spmd

    nc = _get_nc(repeats)
    res = run_bass_kernel_spmd(nc, in_maps, core_ids=list(range(NCORES)))
    LAST_RESULT = res
    return res


def gather(res):
    return np.concatenate([_unshuffle(r["out"]) for r in res.results], axis=0)


def kernel(x, h, W_ih, W_hh, b_ih, b_hh, W_out, b_out):
    in_maps = _host_prep(x, h, W_ih, W_hh, b_ih, b_hh, W_out, b_out)
    res = run(in_maps, repeats=1)
    return gather(res)



# revision 33
# speedup vs baseline: 13.4351x; 13.4351x over previous
"""Trainium2 Bass kernel for the autoregressive GRU decoder.

Reference computation (eval-mode Decoder):
  x0 = x[:, 30, :]                # only element of x ever used
  h0 = h[0]
  for t in 0..29:
      h = GRUCell(x_t, h)         # PyTorch gate layout [r, z, n]
      y_t = h @ W_out.T + b_out
      x_{t+1} = y_t               # linear feedback -> fold into weights
  out = stack(y_t)                # [B, 30, 32]

Because the feedback x_{t+1} = W_out @ h_t + b_out is linear, for t >= 1:
  gi_t = W_ih @ x_t + b_ih = (W_ih @ W_out) @ h_{t-1} + (W_ih @ b_out + b_ih)
so every step t >= 1 is a pure H->H recurrence; weights are folded on the
host and the r/z gates use a single combined matrix (W_hh + W_ih_eff).

Kernel shape notes (v2):
  - state h^T transposed [H=128 partitions, Bc=2048 free], updated in place.
  - batch split into 2 chunks of 1024; each chunk owns 4 PSUM banks
    ([r|z] -> [hn|in] -> y in the freed z banks), so the two chunks'
    serial recurrence chains interleave on the engines, and the 30 steps
    are FULLY UNROLLED (no hardware loop): the tile scheduler overlaps
    steps with point-to-point semaphores instead of loop barriers.
  - elementwise rebalance across engines: h' = z'*n + z*h with the
    off-critical helpers z' = 1-z and t1 = z*h on GpSimd, the PSUM-bound
    u/v gate combines on DVE, transcendentals + y bias-copy on Scalar;
    only 2 ops follow tanh on the recurrence chain.
  - elementwise rebalance across engines: h' = z'*n + z*h with the
    off-critical helpers z' = 1-z and t1 = z*h plus the h' sum on
    GpSimd, PSUM-bound u/v gate combines on DVE, transcendentals and
    the y bias-copy on Scalar; 2 ops follow tanh on the chain.
  - gates and phase-C temporaries in bf16 (DVE 16-bit = 2x); the h'
    accumulation itself stays f32 (bf16 multiplicands, f32 sum), rel
    err ~2.5e-3 vs the 2e-2 gate.
  - y^T = W_out @ h' with f32r operands, into each chunk's freed z
    banks; the whole y tail (matmuls, bias-copy, per-step DMA of a
    double-buffered tile) is emitted inside the NEXT step (software-
    pipelined emission).  NOTE: computing y as W@t2 + W@t1 with a bf16
    W_out stationary (bitcast slice + standalone LdWeights) was ~10%
    faster in CoreSim and numerically correct there, but produced
    garbage on hardware (outputs ~20x too large, consistent with the
    accumulation clear not happening) -- keep y matmuls f32r
    self-loading.
  - f32r for all recurrence matmul operands (full-rate PE).

Timing mode: repeats>1 wraps the whole run in an OUTER HARDWARE loop so
the NEFF size is independent of the repeat count; wall(R2)-wall(R1)
isolates on-device time (python-unrolled repeats scale the instruction
stream and with it the per-call overhead, poisoning the estimate).

Sharding: pure data parallel over batch, 8 cores x 2048, no collectives.
"""

import numpy as np

B, T, I, H, SEQLEN = 16384, 60, 32, 128, 30
STEPS = T - SEQLEN  # 30
NCORES = 8
BC = B // NCORES  # 2048 batch rows per core
CB = BC // 2  # chunk size: 2 pipelined chunks per core
MMN = 512  # matmul moving-operand free-dim limit (one PSUM bank of fp32)

LAST_RESULT = None  # BassKernelResults of the most recent run (for test.py)

_CACHE = {}


def _build(repeats=1):
    from contextlib import ExitStack

    import concourse.bacc as bacc
    import concourse.mybir as mybir
    import concourse.tile as tile

    f32 = mybir.dt.float32
    f32r = mybir.dt.float32r
    bf16 = mybir.dt.bfloat16
    Alu = mybir.AluOpType
    Act = mybir.ActivationFunctionType

    nc = bacc.Bacc()

    # packed inputs (fewer DMAs -> fewer instructions and wait sources):
    # cst  [H, 4H | 2H | I | 8]  = WA | WA0 | WoutT | BIAS(bitcast f32)
    # xw   [I, BC | 3H]          = x0^T | W0
    # h0t  [H, BC]
    CW = 4 * H + 2 * H + I + 8 + I // 2
    dcst = nc.dram_tensor("cst", [H, CW], f32r, kind="ExternalInput")
    dxw = nc.dram_tensor("xw", [I, BC + 3 * H], f32r, kind="ExternalInput")
    dh = nc.dram_tensor("h0t", [H, BC], f32r, kind="ExternalInput")
    dout = nc.dram_tensor("out", [STEPS, I * BC], f32, kind="ExternalOutput")

    with ExitStack() as ctx:
        tc = ctx.enter_context(tile.TileContext(nc))
        const = ctx.enter_context(tc.tile_pool(name="const", bufs=1))
        work = ctx.enter_context(tc.tile_pool(name="work", bufs=1))
        psum = ctx.enter_context(tc.tile_pool(name="psum", bufs=1, space="PSUM"))

        def load_const(dram, shape, name):
            t = const.tile(shape, dram.dtype, tag=name)
            nc.sync.dma_start(out=t[:], in_=dram[:, :])
            return t

        scst = load_const(dcst, [H, CW], "cst")
        sxw = load_const(dxw, [I, BC + 3 * H], "xw")
        hT = const.tile([H, BC], f32r, tag="h")  # the state, updated in place

        sbias = scst[:, 6 * H + I : 6 * H + I + 8].bitcast(f32)
        b_r = sbias[:, 0:1]
        b_z = sbias[:, 1:2]
        b_hn = sbias[:, 2:3]
        b_in = sbias[:, 3:4]
        b0_r = sbias[:, 4:5]
        b0_z = sbias[:, 5:6]
        b0_in = sbias[:, 6:7]
        b_y = sbias[0:I, 7:8]  # b_out padded into rows 0..31
        # bf16 copy of W_out^T for the y matmuls: their moving operands
        # (t1, t2) are bf16, and PE operand dtypes must match
        Wout16 = scst[:, 6 * H + I + 8 : 6 * H + I + 8 + I // 2].bitcast(bf16)

        A_r = scst[:, 0 * H : 1 * H]
        A_z = scst[:, 1 * H : 2 * H]
        A_hn = scst[:, 2 * H : 3 * H]
        A_in = scst[:, 3 * H : 4 * H]
        A0_r = scst[:, 4 * H : 5 * H]
        A0_z = scst[:, 5 * H : 6 * H]
        WoutT = scst[:, 6 * H : 6 * H + I]
        sx0 = sxw[:, 0:BC]
        W0_r = sxw[:, BC + 0 * H : BC + 1 * H]
        W0_z = sxw[:, BC + 1 * H : BC + 2 * H]
        W0_n = sxw[:, BC + 2 * H : BC + 3 * H]

        def mm_pair(dst, A, rhs, extra=None):
            """dst [*, CB] = A.T @ rhs (2 x N=512), + extra (W0_g, x0c)."""
            for q in range(2):
                sl = slice(q * MMN, (q + 1) * MMN)
                d = dst[:, q * MMN : (q + 1) * MMN]
                nc.tensor.matmul(d, A, rhs[:, sl], start=True, stop=extra is None)
                if extra is not None:
                    nc.tensor.matmul(d, extra[0], extra[1][:, sl],
                                     start=False, stop=True)

        def gru_step(first, y_dst, flush_tail, order=(0, 1)):
            """One GRU step over both chunks; y DMA'd to y_dst AP.

            Emission is software-pipelined: the tail of the PREVIOUS step
            (y mms for the late chunk, bias-copy, DMA) is spliced in right
            after this step's phase-A matmuls via flush_tail(), so it
            never blocks the next recurrence iteration in the engine
            streams.  Returns this step's tail closure."""
            P = psum.tile([128, 2 * BC], f32, tag="P", bufs=1)
            # gates and phase-C temporaries in bf16: DVE/GpSimd 16-bit ops
            # run at 2x, and the recurrence precision loss is acceptable
            # (h' itself stays f32; only multiplicands are rounded)
            r_sb = work.tile([128, BC], bf16, tag="r")
            z_sb = work.tile([128, BC], bf16, tag="z")
            zp_sb = work.tile([128, BC], bf16, tag="zp")
            t1_sb = work.tile([128, BC], bf16, tag="t1")
            u_sb = work.tile([128, BC], f32, tag="u")
            n_sb = work.tile([128, BC], bf16, tag="n")
            y_sb = work.tile([I, BC], f32, tag="y", bufs=2)

            if first:
                cb_r, cb_z, cb_in = b0_r, b0_z, b0_in
                Ar, Az = A0_r, A0_z
            else:
                cb_r, cb_z, cb_in = b_r, b_z, b_in
                Ar, Az = A_r, A_z

            CS = [slice(0, CB), slice(CB, BC)]  # chunk batch slices
            hc = [hT[:, s] for s in CS]
            xc = [sx0[:, s] for s in CS] if first else [None, None]
            rP = [P[:, 0:CB], P[:, 2 * CB : 3 * CB]]
            zP = [P[:, CB : 2 * CB], P[:, 3 * CB : 4 * CB]]

            # phase A: r/z gate matmuls, grouped by gate (one stationary
            # load serves both chunks); chunk ops interleave on each
            # engine so the two recurrence chains pipeline
            for c in order:
                mm_pair(rP[c], Ar, hc[c], (W0_r, xc[c]) if first else None)
            # previous step's y tail is spliced in BEFORE the z matmuls:
            # y lives in the Z banks, so its bias-copies must be emitted
            # before this step's zA mms overwrite them (program order is
            # dependency order for the tracker); rA above starts clean on
            # the critical h'->rA edge with no stationary eviction
            flush_tail()
            for c in order:
                mm_pair(zP[c], Az, hc[c], (W0_z, xc[c]) if first else None)
            # both r sigmoids before the z ones: r gates the critical
            # u-chain of each chunk, z only feeds the off-critical helpers
            for c in order:
                nc.scalar.activation(r_sb[:, CS[c]], rP[c], Act.Sigmoid, bias=cb_r)
            for c in order:
                nc.scalar.activation(z_sb[:, CS[c]], zP[c], Act.Sigmoid, bias=cb_z)

            # phase B: hn/in reuse the chunk's banks after the sigmoids
            for c in order:
                mm_pair(rP[c], A_hn, hc[c])
            for c in order:
                if first:
                    mm_pair(zP[c], W0_n, xc[c])
                else:
                    mm_pair(zP[c], A_in, hc[c])

            for c in order:
                # off-critical helpers on GpSimd: z' = 1-z, t1 = z*h
                nc.gpsimd.tensor_scalar(
                    zp_sb[:, CS[c]], z_sb[:, CS[c]], -1.0, 1.0, Alu.mult, Alu.add
                )
                nc.gpsimd.tensor_tensor(
                    t1_sb[:, CS[c]], z_sb[:, CS[c]], hc[c], Alu.mult
                )
                # u = (g_hn + b_hn) * r ; v = (u + b_in) + g_in (in place)
                nc.vector.scalar_tensor_tensor(
                    u_sb[:, CS[c]], rP[c], b_hn, r_sb[:, CS[c]], Alu.add, Alu.mult
                )
                nc.vector.scalar_tensor_tensor(
                    u_sb[:, CS[c]], zP[c], cb_in, u_sb[:, CS[c]], Alu.add, Alu.add
                )
                nc.scalar.activation(n_sb[:, CS[c]], u_sb[:, CS[c]], Act.Tanh)

            for c in order:
                # h' = z'*n + z*h  (2-op critical chain after tanh; the
                # multiply runs bf16 2x on DVE, the add sums in f32 on
                # GpSimd to keep the state and DVE free)
                nc.vector.tensor_tensor(
                    n_sb[:, CS[c]], zp_sb[:, CS[c]], n_sb[:, CS[c]], Alu.mult
                )
                nc.gpsimd.tensor_tensor(hc[c], n_sb[:, CS[c]], t1_sb[:, CS[c]], Alu.add)

            # phase D: y^T = W_out @ h' = W@t2 + W@t1 (matmul distributes
            # over h' = t2 + t1), so y never depends on the h' add and the
            # tail decouples from the recurrence chain.  bf16 moving
            # operands allow the full 1024-wide chunk in one matmul.
            # Deferred into the NEXT step's emission (flush_tail).
            def tail():
                for c in order:
                    mm_pair(zP[c][0:I, :], WoutT, hc[c])
                for c in order:
                    nc.scalar.activation(
                        y_sb[:, CS[c]], zP[c][0:I, :], Act.Identity, bias=b_y
                    )
                nc.sync.dma_start(out=y_dst, in_=y_sb[:, None, :])

            return tail

        def one_run():
            nc.sync.dma_start(out=hT[:], in_=dh[:, :])
            tail = gru_step(
                True, dout[0:1, :].rearrange("o (p f) -> p o f", p=I),
                lambda: None,
            )
            for t in range(1, STEPS):
                tail = gru_step(
                    False, dout[t : t + 1, :].rearrange("o (p f) -> p o f", p=I),
                    tail, (0, 1) if t % 2 == 0 else (1, 0),
                )
            tail()

        if repeats == 1:
            one_run()
        else:
            # timing mode: outer HARDWARE loop -> NEFF size is independent
            # of the repeat count (see module docstring)
            with tc.For_i(0, repeats, 1):
                one_run()

    return nc


def _host_prep(x, h, W_ih, W_hh, b_ih, b_hh, W_out, b_out):
    """Fold weights on the host (float64 for exactness), build per-core maps."""
    x = np.asarray(x, dtype=np.float32)
    h = np.asarray(h, dtype=np.float32)
    W_ih = np.asarray(W_ih, dtype=np.float64)
    W_hh = np.asarray(W_hh, dtype=np.float64)
    b_ih = np.asarray(b_ih, dtype=np.float64)
    b_hh = np.asarray(b_hh, dtype=np.float64)
    W_out = np.asarray(W_out, dtype=np.float64)
    b_out = np.asarray(b_out, dtype=np.float64)

    W_ih_eff = W_ih @ W_out  # [3H, H]
    b_ih_eff = W_ih @ b_out + b_ih  # [3H]

    def cvt(a):
        return np.ascontiguousarray(a, dtype=np.float32)

    WA = cvt(
        np.concatenate(
            [
                (W_hh[0:H] + W_ih_eff[0:H]).T,
                (W_hh[H : 2 * H] + W_ih_eff[H : 2 * H]).T,
                W_hh[2 * H : 3 * H].T,
                W_ih_eff[2 * H : 3 * H].T,
            ],
            axis=1,
        )
    )  # [H, 4H]
    WA0 = cvt(np.concatenate([W_hh[0:H].T, W_hh[H : 2 * H].T], axis=1))
    W0 = cvt(
        np.concatenate([W_ih[0:H].T, W_ih[H : 2 * H].T, W_ih[2 * H : 3 * H].T], axis=1)
    )  # [I, 3H]
    WoutT = cvt(W_out.T)  # [H, I]
    bx = np.zeros(H)
    bx[0:I] = b_out
    BIAS = cvt(
        np.stack(
            [
                b_hh[0:H] + b_ih_eff[0:H],
                b_hh[H : 2 * H] + b_ih_eff[H : 2 * H],
                b_hh[2 * H : 3 * H],
                b_ih_eff[2 * H : 3 * H],
                b_hh[0:H] + b_ih[0:H],
                b_hh[H : 2 * H] + b_ih[H : 2 * H],
                b_ih[2 * H : 3 * H],
                bx,
            ],
            axis=1,
        )
    )  # [H, 8]

    x0T = cvt(x[:, SEQLEN, :].T)  # [I, B]
    h0T = cvt(h[0].T)  # [H, B]

    u = WoutT.view(np.uint32)  # bf16(RNE) pack of W_out^T, pairs per f32 col
    W16 = ((u + 0x7FFF + ((u >> 16) & 1)) >> 16).astype(np.uint32)
    W16 = (W16[:, 0::2] | (W16[:, 1::2] << 16)).view(np.float32)
    CST = np.concatenate([WA, WA0, WoutT, BIAS, W16], axis=1)  # [H, CW]
    in_maps = []
    for core in range(NCORES):
        cs = slice(core * BC, (core + 1) * BC)
        in_maps.append(
            {
                "cst": CST,
                "xw": np.concatenate(
                    [np.ascontiguousarray(x0T[:, cs]), W0], axis=1
                ),
                "h0t": np.ascontiguousarray(h0T[:, cs]),
            }
        )
    return in_maps


def _unshuffle(out_dev):
    """[STEPS, I*BC] device layout -> [BC, STEPS, I]."""
    x = out_dev.reshape(STEPS, I, BC)
    return np.ascontiguousarray(x.transpose(2, 0, 1))


def _get_nc(repeats=1):
    key = repeats
    if key not in _CACHE:
        nc = _build(repeats)
        # Bacc needs explicit finalize (wait-splitting, reg alloc);
        # run_bass_via_pjrt serializes the module as-is.
        nc.finalize()
        _CACHE[key] = nc
    return _CACHE[key]


def run(in_maps, repeats=1):
    global LAST_RESULT
    from concourse.bass_utils import run_bass_kernel_spmd

    nc = _get_nc(repeats)
    res = run_bass_kernel_spmd(nc, in_maps, core_ids=list(range(NCORES)))
    LAST_RESULT = res
    return res


def gather(res):
    return np.concatenate([_unshuffle(r["out"]) for r in res.results], axis=0)


def kernel(x, h, W_ih, W_hh, b_ih, b_hh, W_out, b_out):
    in_maps = _host_prep(x, h, W_ih, W_hh, b_ih, b_hh, W_out, b_out)
    res = run(in_maps, repeats=1)
    return gather(res)


# revision 43
# speedup vs baseline: 21.1129x; 1.5715x over previous
"""Trainium2 Bass kernel for the autoregressive GRU decoder.

Reference computation (eval-mode Decoder):
  x0 = x[:, 30, :]                # only element of x ever used
  h0 = h[0]
  for t in 0..29:
      h = GRUCell(x_t, h)         # PyTorch gate layout [r, z, n]
      y_t = h @ W_out.T + b_out
      x_{t+1} = y_t               # linear feedback -> fold into weights
  out = stack(y_t)                # [B, 30, 32]

Because the feedback x_{t+1} = W_out @ h_t + b_out is linear, for t >= 1:
  gi_t = W_ih @ x_t + b_ih = (W_ih @ W_out) @ h_{t-1} + (W_ih @ b_out + b_ih)
so every step t >= 1 is a pure H->H recurrence; weights are folded on the
host and the r/z gates use a single combined matrix (W_hh + W_ih_eff).

Kernel shape notes (v2):
  - state h^T transposed [H=128 partitions, Bc=2048 free], updated in place.
  - batch split into 2 chunks of 1024; each chunk owns 4 PSUM banks
    ([r|z] -> [hn|in] -> y in the freed z banks), so the two chunks'
    serial recurrence chains interleave on the engines, and the 30 steps
    are FULLY UNROLLED (no hardware loop): the tile scheduler overlaps
    steps with point-to-point semaphores instead of loop barriers.
  - elementwise rebalance across engines: h' = z'*n + z*h with the
    off-critical helpers z' = 1-z and t1 = z*h plus the h' sum on
    GpSimd, PSUM-bound u/v gate combines on DVE, transcendentals and
    the y bias-copy on Scalar; 2 ops follow tanh on the chain.
  - gates and phase-C temporaries in bf16 (DVE 16-bit = 2x); the h'
    accumulation itself stays f32 (bf16 multiplicands, f32 sum), rel
    err ~2.5e-3 vs the 2e-2 gate.
  - y^T = W_out @ h' computed as W@t2 + W@t1 (matmul distributes over
    h' = t2 + t1, both already in SBUF as bf16), so the output path
    never touches the recurrence chain; y is PACKED [4*I, 512] via four
    zero-padded stationaries (W_out^T placed at column offset q*32)
    accumulating into one psum bank, so the PSUM->SBUF bias-copy is a
    single 512-wide op with all 128 Scalar lanes active; the whole y
    tail (matmuls, bias-copy, per-step DMA of a double-buffered tile)
    is emitted inside the NEXT step (software-pipelined emission).
    The bf16 stationaries MUST be their own bf16 dram tensors: sourcing
    them from a .bitcast(bf16) slice of the f32r const tile produced
    garbage on hardware (~20x-too-large outputs, accumulation clear
    failing) despite computing correctly in CoreSim.
  - f32r for all recurrence matmul operands (full-rate PE).

Timing mode: repeats>1 wraps the whole run in an OUTER HARDWARE loop so
the NEFF size is independent of the repeat count; wall(R2)-wall(R1)
isolates on-device time (python-unrolled repeats scale the instruction
stream and with it the per-call overhead, poisoning the estimate).

Sharding: pure data parallel over batch, 8 cores x 2048, no collectives.
"""

import numpy as np

B, T, I, H, SEQLEN = 16384, 60, 32, 128, 30
STEPS = T - SEQLEN  # 30
NCORES = 8
BC = B // NCORES  # 2048 batch rows per core
CB = BC // 2  # chunk size: 2 pipelined chunks per core
MMN = 512  # matmul moving-operand free-dim limit (one PSUM bank of fp32)

LAST_RESULT = None  # BassKernelResults of the most recent run (for test.py)

_CACHE = {}


def _build(repeats=1):
    from contextlib import ExitStack

    import concourse.bacc as bacc
    import concourse.mybir as mybir
    import concourse.tile as tile

    f32 = mybir.dt.float32
    f32r = mybir.dt.float32r
    bf16 = mybir.dt.bfloat16
    Alu = mybir.AluOpType
    Act = mybir.ActivationFunctionType

    nc = bacc.Bacc()

    # packed inputs (fewer DMAs -> fewer instructions and wait sources):
    # cst  [H, 4H | 2H | I | 8]  = WA | WA0 | WoutT | BIAS(bitcast f32)
    # xw   [I, BC | 3H]          = x0^T | W0
    # h0t  [H, BC]
    CW = 4 * H + 2 * H + I + 8 + I // 2
    dcst = nc.dram_tensor("cst", [H, CW], f32r, kind="ExternalInput")
    dxw = nc.dram_tensor("xw", [I, BC + 3 * H], f32r, kind="ExternalInput")
    dh = nc.dram_tensor("h0t", [H, BC], f32r, kind="ExternalInput")
    dw16 = nc.dram_tensor("w16", [H, I], bf16, kind="ExternalInput")
    dw4 = nc.dram_tensor("w4", [H, 4 * H], bf16, kind="ExternalInput")
    dout = nc.dram_tensor("out", [STEPS, I * BC], f32, kind="ExternalOutput")

    with ExitStack() as ctx:
        tc = ctx.enter_context(tile.TileContext(nc))
        const = ctx.enter_context(tc.tile_pool(name="const", bufs=1))
        work = ctx.enter_context(tc.tile_pool(name="work", bufs=1))
        psum = ctx.enter_context(tc.tile_pool(name="psum", bufs=1, space="PSUM"))

        def load_const(dram, shape, name):
            t = const.tile(shape, dram.dtype, tag=name)
            nc.sync.dma_start(out=t[:], in_=dram[:, :])
            return t

        scst = load_const(dcst, [H, CW], "cst")
        sw16 = load_const(dw16, [H, I], "w16")
        sw4 = load_const(dw4, [H, 4 * H], "w4")
        sxw = load_const(dxw, [I, BC + 3 * H], "xw")
        hT = const.tile([H, BC], f32r, tag="h")  # the state, updated in place

        sbias = scst[:, 6 * H + I : 6 * H + I + 8].bitcast(f32)
        b_r = sbias[:, 0:1]
        b_z = sbias[:, 1:2]
        b_hn = sbias[:, 2:3]
        b_in = sbias[:, 3:4]
        b0_r = sbias[:, 4:5]
        b0_z = sbias[:, 5:6]
        b0_in = sbias[:, 6:7]
        b_y = sbias[:, 7:8]  # b_out tiled x4 (y is packed [4*I, 512])
        # bf16 copy of W_out^T for the y matmuls: their moving operands
        # (t1, t2) are bf16, and PE operand dtypes must match
        Wout16 = scst[:, 6 * H + I + 8 : 6 * H + I + 8 + I // 2].bitcast(bf16)

        A_r = scst[:, 0 * H : 1 * H]
        A_z = scst[:, 1 * H : 2 * H]
        A_hn = scst[:, 2 * H : 3 * H]
        A_in = scst[:, 3 * H : 4 * H]
        A0_r = scst[:, 4 * H : 5 * H]
        A0_z = scst[:, 5 * H : 6 * H]
        WoutT = scst[:, 6 * H : 6 * H + I]
        sx0 = sxw[:, 0:BC]
        W0_r = sxw[:, BC + 0 * H : BC + 1 * H]
        W0_z = sxw[:, BC + 1 * H : BC + 2 * H]
        W0_n = sxw[:, BC + 2 * H : BC + 3 * H]

        def mm_pair(dst, A, rhs, extra=None):
            """dst [*, CB] = A.T @ rhs (2 x N=512), + extra (W0_g, x0c)."""
            for q in range(2):
                sl = slice(q * MMN, (q + 1) * MMN)
                d = dst[:, q * MMN : (q + 1) * MMN]
                nc.tensor.matmul(d, A, rhs[:, sl], start=True, stop=extra is None)
                if extra is not None:
                    nc.tensor.matmul(d, extra[0], extra[1][:, sl],
                                     start=False, stop=True)

        def gru_step(first, y_dst, flush_tail, order=(0, 1)):
            """One GRU step over both chunks; y DMA'd to y_dst AP.

            Emission is software-pipelined: the tail of the PREVIOUS step
            (y mms for the late chunk, bias-copy, DMA) is spliced in right
            after this step's phase-A matmuls via flush_tail(), so it
            never blocks the next recurrence iteration in the engine
            streams.  Returns this step's tail closure."""
            P = psum.tile([128, 2 * BC], f32, tag="P", bufs=1)
            # gates and phase-C temporaries in bf16: DVE/GpSimd 16-bit ops
            # run at 2x, and the recurrence precision loss is acceptable
            # (h' itself stays f32; only multiplicands are rounded)
            r_sb = work.tile([128, BC], bf16, tag="r")
            z_sb = work.tile([128, BC], bf16, tag="z")
            zp_sb = work.tile([128, BC], bf16, tag="zp")
            t1_sb = work.tile([128, BC], bf16, tag="t1")
            u_sb = work.tile([128, BC], f32, tag="u")
            n_sb = work.tile([128, BC], bf16, tag="n")
            y_sb = work.tile([128, MMN], f32, tag="y", bufs=2)

            if first:
                cb_r, cb_z, cb_in = b0_r, b0_z, b0_in
                Ar, Az = A0_r, A0_z
            else:
                cb_r, cb_z, cb_in = b_r, b_z, b_in
                Ar, Az = A_r, A_z

            CS = [slice(0, CB), slice(CB, BC)]  # chunk batch slices
            hc = [hT[:, s] for s in CS]
            xc = [sx0[:, s] for s in CS] if first else [None, None]
            rP = [P[:, 0:CB], P[:, 2 * CB : 3 * CB]]
            zP = [P[:, CB : 2 * CB], P[:, 3 * CB : 4 * CB]]

            # phase A: r/z gate matmuls, grouped by gate (one stationary
            # load serves both chunks); chunk ops interleave on each
            # engine so the two recurrence chains pipeline
            for c in order:
                mm_pair(rP[c], Ar, hc[c], (W0_r, xc[c]) if first else None)
            # previous step's y tail is spliced in BEFORE the z matmuls:
            # y lives in the Z banks, so its bias-copies must be emitted
            # before this step's zA mms overwrite them (program order is
            # dependency order for the tracker); rA above starts clean on
            # the critical h'->rA edge with no stationary eviction
            flush_tail()
            for c in order:
                mm_pair(zP[c], Az, hc[c], (W0_z, xc[c]) if first else None)
            # both r sigmoids before the z ones: r gates the critical
            # u-chain of each chunk, z only feeds the off-critical helpers
            for c in order:
                nc.scalar.activation(r_sb[:, CS[c]], rP[c], Act.Sigmoid, bias=cb_r)
            for c in order:
                nc.scalar.activation(z_sb[:, CS[c]], zP[c], Act.Sigmoid, bias=cb_z)

            # phase B: hn/in reuse the chunk's banks after the sigmoids
            for c in order:
                mm_pair(rP[c], A_hn, hc[c])
            for c in order:
                if first:
                    mm_pair(zP[c], W0_n, xc[c])
                else:
                    mm_pair(zP[c], A_in, hc[c])

            for c in order:
                # off-critical helpers on GpSimd: z' = 1-z, t1 = z*h
                nc.gpsimd.tensor_scalar(
                    zp_sb[:, CS[c]], z_sb[:, CS[c]], -1.0, 1.0, Alu.mult, Alu.add
                )
                nc.gpsimd.tensor_tensor(
                    t1_sb[:, CS[c]], z_sb[:, CS[c]], hc[c], Alu.mult
                )
                # u = (g_hn + b_hn) * r ; v = (u + b_in) + g_in (in place)
                nc.vector.scalar_tensor_tensor(
                    u_sb[:, CS[c]], rP[c], b_hn, r_sb[:, CS[c]], Alu.add, Alu.mult
                )
                nc.vector.scalar_tensor_tensor(
                    u_sb[:, CS[c]], zP[c], cb_in, u_sb[:, CS[c]], Alu.add, Alu.add
                )
                for q in range(2):
                    s = slice(c * CB + q * MMN, c * CB + (q + 1) * MMN)
                    nc.scalar.activation(n_sb[:, s], u_sb[:, s], Act.Tanh)

            for c in order:
                # h' = z'*n + z*h  (2-op critical chain after tanh; the
                # multiply runs bf16 2x on DVE, the add sums in f32 on
                # GpSimd to keep the state and DVE free)
                for q in range(2):
                    s = slice(c * CB + q * MMN, c * CB + (q + 1) * MMN)
                    nc.vector.tensor_tensor(n_sb[:, s], zp_sb[:, s], n_sb[:, s],
                                            Alu.mult)
                nc.gpsimd.tensor_tensor(hc[c], n_sb[:, CS[c]], t1_sb[:, CS[c]], Alu.add)

            # phase D: y^T = W_out @ h' = W@t2 + W@t1 (matmul distributes
            # over h' = t2 + t1), so y never depends on the h' add and the
            # tail decouples from the recurrence chain.  bf16 moving
            # operands allow the full 1024-wide chunk in one matmul.
            # Deferred into the NEXT step's emission (flush_tail).
            # y packed [4*I, 512]: batch quarter q lands on partitions
            # q*32..(q+1)*32 of ONE psum bank via a zero-padded stationary
            # (W_out^T in columns q*32..) -- all 8 matmuls accumulate into
            # the same bank (zeros elsewhere are harmless), and the bias-
            # copy is one 512-wide op with all 128 ACT lanes active
            y4P = P[:, 4 * CB - MMN : 4 * CB]  # bank 7, freed by stt v (c1)

            def tail():
                for q in range(4):
                    s = slice(q * MMN, (q + 1) * MMN)
                    W4q = sw4[:, q * H : (q + 1) * H]
                    nc.tensor.matmul(y4P, W4q, t1_sb[:, s],
                                     start=q == 0, stop=False)
                    nc.tensor.matmul(y4P, W4q, n_sb[:, s],
                                     start=False, stop=q == 3)
                nc.scalar.activation(y_sb[:], y4P, Act.Identity, bias=b_y)
                nc.sync.dma_start(out=y_dst, in_=y_sb[:, None, :])

            return tail

        def one_run():
            nc.sync.dma_start(out=hT[:], in_=dh[:, :])
            tail = gru_step(
                True, dout[0:1, :].rearrange("o (p f) -> p o f", p=128),
                lambda: None,
            )
            for t in range(1, STEPS):
                tail = gru_step(
                    False, dout[t : t + 1, :].rearrange("o (p f) -> p o f", p=128),
                    tail, (0, 1) if t % 2 == 0 else (1, 0),
                )
            tail()

        if repeats == 1:
            one_run()
        else:
            # timing mode: outer HARDWARE loop -> NEFF size is independent
            # of the repeat count (see module docstring)
            with tc.For_i(0, repeats, 1):
                one_run()

    return nc


def _host_prep(x, h, W_ih, W_hh, b_ih, b_hh, W_out, b_out):
    """Fold weights on the host (float64 for exactness), build per-core maps."""
    x = np.asarray(x, dtype=np.float32)
    h = np.asarray(h, dtype=np.float32)
    W_ih = np.asarray(W_ih, dtype=np.float64)
    W_hh = np.asarray(W_hh, dtype=np.float64)
    b_ih = np.asarray(b_ih, dtype=np.float64)
    b_hh = np.asarray(b_hh, dtype=np.float64)
    W_out = np.asarray(W_out, dtype=np.float64)
    b_out = np.asarray(b_out, dtype=np.float64)

    W_ih_eff = W_ih @ W_out  # [3H, H]
    b_ih_eff = W_ih @ b_out + b_ih  # [3H]

    def cvt(a):
        return np.ascontiguousarray(a, dtype=np.float32)

    WA = cvt(
        np.concatenate(
            [
                (W_hh[0:H] + W_ih_eff[0:H]).T,
                (W_hh[H : 2 * H] + W_ih_eff[H : 2 * H]).T,
                W_hh[2 * H : 3 * H].T,
                W_ih_eff[2 * H : 3 * H].T,
            ],
            axis=1,
        )
    )  # [H, 4H]
    WA0 = cvt(np.concatenate([W_hh[0:H].T, W_hh[H : 2 * H].T], axis=1))
    W0 = cvt(
        np.concatenate([W_ih[0:H].T, W_ih[H : 2 * H].T, W_ih[2 * H : 3 * H].T], axis=1)
    )  # [I, 3H]
    WoutT = cvt(W_out.T)  # [H, I]
    bx = np.tile(b_out, 4)  # y packed [4*I, 512]
    BIAS = cvt(
        np.stack(
            [
                b_hh[0:H] + b_ih_eff[0:H],
                b_hh[H : 2 * H] + b_ih_eff[H : 2 * H],
                b_hh[2 * H : 3 * H],
                b_ih_eff[2 * H : 3 * H],
                b_hh[0:H] + b_ih[0:H],
                b_hh[H : 2 * H] + b_ih[H : 2 * H],
                b_ih[2 * H : 3 * H],
                bx,
            ],
            axis=1,
        )
    )  # [H, 8]

    x0T = cvt(x[:, SEQLEN, :].T)  # [I, B]
    h0T = cvt(h[0].T)  # [H, B]

    u = WoutT.view(np.uint32)  # bf16(RNE) pack of W_out^T, pairs per f32 col
    W16 = ((u + 0x7FFF + ((u >> 16) & 1)) >> 16).astype(np.uint32)
    W16p = (W16[:, 0::2] | (W16[:, 1::2] << 16)).view(np.float32)
    CST = np.concatenate([WA, WA0, WoutT, BIAS, W16p], axis=1)  # [H, CW]
    import ml_dtypes

    W16n = (W16.astype(np.uint32) << 16).view(np.float32).astype(ml_dtypes.bfloat16)
    W4 = np.zeros((H, 4 * H), dtype=ml_dtypes.bfloat16)
    for q in range(4):
        W4[:, q * H + q * I : q * H + (q + 1) * I] = W16n
    in_maps = []
    for core in range(NCORES):
        cs = slice(core * BC, (core + 1) * BC)
        in_maps.append(
            {
                "cst": CST,
                "xw": np.concatenate(
                    [np.ascontiguousarray(x0T[:, cs]), W0], axis=1
                ),
                "h0t": np.ascontiguousarray(h0T[:, cs]),
                "w16": W16n,
                "w4": W4,
            }
        )
    return in_maps


def _unshuffle(out_dev):
    """[STEPS, 4*I*512] packed device layout -> [BC, STEPS, I]."""
    x = out_dev.reshape(STEPS, 4, I, BC // 4)
    return np.ascontiguousarray(x.transpose(1, 3, 0, 2).reshape(BC, STEPS, I))


def _get_nc(repeats=1):
    key = repeats
    if key not in _CACHE:
        nc = _build(repeats)
        # Bacc needs explicit finalize (wait-splitting, reg alloc);
        # run_bass_via_pjrt serializes the module as-is.
        nc.finalize()
        _CACHE[key] = nc
    return _CACHE[key]


def run(in_maps, repeats=1):
    global LAST_RESULT
    from concourse.bass_utils import run_bass_kernel_spmd

    nc = _get_nc(repeats)
    res = run_bass_kernel_spmd(nc, in_maps, core_ids=list(range(NCORES)))
    LAST_RESULT = res
    return res


def gather(res):
    return np.concatenate([_unshuffle(r["out"]) for r in res.results], axis=0)


def kernel(x, h, W_ih, W_hh, b_ih, b_hh, W_out, b_out):
    in_maps = _host_prep(x, h, W_ih, W_hh, b_ih, b_hh, W_out, b_out)
    res = run(in_maps, repeats=1)
    return gather(res)
